# revision 13
# baseline (speedup 1.0000x reference)
"""Trainium2 Bass kernel for nn_MultiHeadAttention_67731634258682.

MHA: B=2, S=8192, D=1024, H=16 heads (depth 64).
Sharding over 8 cores: core c -> (batch b = c//4, head-group g = c%4).
Each core computes its 4 heads end-to-end plus a row-parallel partial of
the output projection; the host sums the 4 partials per batch.

v2 design (vs v1 baseline at 3.09 ms):
  - Everything SBUF-resident: K^T/Q^T (2 pairs x [128, S]), V' (2 pairs x
    [128, nkc, 130] with ones columns), pair-0 O^T [128, S]. Projections
    evacuate straight into the resident tiles (no DRAM scratch round-trip).
  - Exp split across engines: ~2/3 of (kc) slots on ScalarE (exact Exp,
    scale folded), ~1/3 on DVE via a Schraudolph bit-trick: bf16 bits =
    int16(round(A*logit + B)), one fused tensor_scalar per slot. Rel-err
    cost measured in simulation: 6.0e-3 -> 1.2e-2 (gate 2e-2).
  - Software-pipelined emission: per slot emit exp(i), lg(i+2), pv(i) so
    the in-order PE stream never head-of-line blocks on an exp; epilogue /
    Wo / next-tile q-projection are chopped into small chunks emitted
    between slots to keep the PE continuously busy (p-state ramp to 2.4
    GHz requires gapless execution).
  - DMA issue moved off ScalarE entirely (sync for loads, gpsimd for Y).
"""

import os
import sys
import numpy as np

for _p in ("/opt/trn_rl_repo", "/root/.axon_site/_ro/trn_rl_repo"):
    if os.path.isdir(_p) and _p not in sys.path:
        sys.path.append(_p)

import concourse.bass as bass
import concourse.mybir as mybir
from concourse import bacc, tile
from concourse.bass import ts, ds
from concourse.masks import make_identity
from concourse.bass_utils import run_bass_kernel_spmd

F32 = mybir.dt.float32
BF16 = mybir.dt.bfloat16
I16 = mybir.dt.int16

B, S, D = 2, 8192, 1024
H = 16
DEPTH = 64          # head dim
G = 4               # head groups (one per core within a batch)
HPG = 4             # heads per group
DG = HPG * DEPTH    # 256 features per group
QT = 512            # q tile
KC = 128            # k chunk (matmul contraction tile)
NDC = D // 128      # 8 contraction chunks for projections

AFT = mybir.ActivationFunctionType
ALU = mybir.AluOpType

SCALE = 0.125                                  # 1/sqrt(64)
SCH_A = SCALE * np.log2(np.e) * 128.0          # schraudolph multiplier
SCH_B0 = 127.0 * 128.0                         # exponent bias in bf16 bits


def build_program(seq=S, dve_num=1, dve_den=3, boff=-7.4):
    """Build the per-core Bass program. Returns the compiled Bacc object."""
    assert seq % QT == 0
    nqt = seq // QT
    nkc = seq // KC
    nsc = seq // QT
    dt = BF16

    nc = bacc.Bacc("TRN2", target_bir_lowering=False, debug=False,
                   enable_asserts=False, num_devices=8)

    # ---- external I/O ----
    qT = nc.dram_tensor("qT", [D, seq], dt, kind="ExternalInput").ap()
    kT = nc.dram_tensor("kT", [D, seq], dt, kind="ExternalInput").ap()
    vT = nc.dram_tensor("vT", [D, seq], dt, kind="ExternalInput").ap()
    Wq = nc.dram_tensor("Wq", [D, DG], dt, kind="ExternalInput").ap()
    Wk = nc.dram_tensor("Wk", [D, DG], dt, kind="ExternalInput").ap()
    Wv = nc.dram_tensor("Wv", [D, DG], dt, kind="ExternalInput").ap()
    Wo = nc.dram_tensor("Wo", [DG, D], dt, kind="ExternalInput").ap()
    bq = nc.dram_tensor("bq", [DG, 1], F32, kind="ExternalInput").ap()
    bk = nc.dram_tensor("bk", [DG, 1], F32, kind="ExternalInput").ap()
    bv = nc.dram_tensor("bv", [1, DG], dt, kind="ExternalInput").ap()
    Y = nc.dram_tensor("Y", [seq, D], F32, kind="ExternalOutput").ap()

    with tile.TileContext(nc) as tc:
        from contextlib import ExitStack
        ctx = ExitStack()
        with ctx:
            const = ctx.enter_context(tc.tile_pool(name="const", bufs=1))
            res = ctx.enter_context(tc.tile_pool(name="res", bufs=1))
            xin = ctx.enter_context(tc.tile_pool(name="xin", bufs=3))
            ppool = ctx.enter_context(tc.tile_pool(name="ppool", bufs=3))
            epi = ctx.enter_context(tc.tile_pool(name="epi", bufs=4))
            otp = ctx.enter_context(tc.tile_pool(name="otp", bufs=3))
            ypool = ctx.enter_context(tc.tile_pool(name="ypool", bufs=3))
            # One shared PSUM ring: 3 slots of 2 banks each (tag "lg") serve
            # the logits tiles AND all small PE outputs (transposes, Wo
            # accumulator, projection accumulator); pv pins the last 2 banks.
            ps_logit = ctx.enter_context(
                tc.tile_pool(name="ps_logit", bufs=3, space="PSUM"))
            ps_pv = ctx.enter_context(
                tc.tile_pool(name="ps_pv", bufs=1, space="PSUM"))

            def ps_tile(shape, name):
                return ps_logit.tile(shape, F32, tag="lg", bufs=3, name=name)

            # ---- constants ----
            ident = const.tile([128, 128], F32, tag="ident")
            make_identity(nc, ident[:])
            ones_f32 = const.tile([128, 128], F32, tag="ones_f32")
            nc.any.memset(ones_f32[:], 1.0)
            ones_row = const.tile([1, 128], dt, tag="ones_row")
            nc.vector.tensor_copy(ones_row[:], ones_f32[0:1, :])
            ident_bf = const.tile([128, 128], dt, tag="ident_bf")
            nc.vector.tensor_copy(ident_bf[:], ident[:])

            wq_sb = [const.tile([128, DG], dt, tag=f"wq{dc}", name=f"wq{dc}")
                     for dc in range(NDC)]
            wk_sb = [const.tile([128, DG], dt, tag=f"wk{dc}", name=f"wk{dc}")
                     for dc in range(NDC)]
            wv_sb = [const.tile([128, DG], dt, tag=f"wv{dc}", name=f"wv{dc}")
                     for dc in range(NDC)]
            for dc in range(NDC):
                nc.sync.dma_start(wq_sb[dc][:], Wq[ts(dc, 128), :])
                nc.sync.dma_start(wk_sb[dc][:], Wk[ts(dc, 128), :])
                nc.sync.dma_start(wv_sb[dc][:], Wv[ts(dc, 128), :])
            wo_sb = [const.tile([128, D], dt, tag=f"wo{i}", name=f"wo{i}")
                     for i in range(2)]
            for i in range(2):
                nc.sync.dma_start(wo_sb[i][:], Wo[ts(i, 128), :])
            bq_sb = [const.tile([128, 1], F32, tag=f"bq{i}", name=f"bq{i}")
                     for i in range(2)]
            bk_sb = [const.tile([128, 1], F32, tag=f"bk{i}", name=f"bk{i}")
                     for i in range(2)]
            for i in range(2):
                nc.sync.dma_start(bq_sb[i][:], bq[ts(i, 128), :])
                nc.sync.dma_start(bk_sb[i][:], bk[ts(i, 128), :])
            bv_sb = const.tile([1, DG], dt, tag="bv_sb")
            nc.sync.dma_start(bv_sb[:], bv[:, :])

            # ---- resident tensors (persist for the whole kernel) ----
            ktr = [res.tile([128, seq], dt, tag=f"ktr{p}", name=f"ktr{p}")
                   for p in range(2)]
            qtr = [res.tile([128, seq], dt, tag=f"qtr{p}", name=f"qtr{p}")
                   for p in range(2)]
            VP_W = 2 * (DEPTH + 1)  # per-pair per-chunk: 2 heads x [V_h|1]
            vpr = [res.tile([128, nkc, VP_W], dt, tag=f"vpr{p}",
                            name=f"vpr{p}") for p in range(2)]
            ot0 = res.tile([128, seq], dt, tag="ot0", name="ot0")
            # ones columns of V' are constant: write once.
            for p in range(2):
                vh = vpr[p][:].rearrange("p k (h x) -> p k h x", x=DEPTH + 1)
                nc.vector.memset(vh[:, :, :, DEPTH:DEPTH + 1], 1.0)

            # ---- projection emitters ----
            def load_x(src, sc, eng):
                xt = xin.tile([128, NDC, QT], dt, tag="xt", name="xt")
                rr = src.rearrange("(c p) s -> p c s", p=128)
                eng.dma_start(xt[:], rr[:, :, ts(sc, QT)])
                return xt

            def qkproj_emit(sc, xt, w_sb, b_sb, dst, f):
                ps = ps_tile([128, QT], "ps")
                for dc in range(NDC):
                    nc.tensor.matmul(
                        ps[:], w_sb[dc][:, ts(f, 128)], xt[:, dc, :],
                        start=(dc == 0), stop=(dc == NDC - 1))
                nc.vector.tensor_scalar_add(
                    dst[f][:, ts(sc, QT)], ps[:], b_sb[f][:])

            def vproj_emit(sc, xt, sub):
                ps = ps_tile([128, DG], "ps")
                for dc in range(NDC):
                    nc.tensor.matmul(
                        ps[:], xt[:, dc, ts(sub, 128)], wv_sb[dc][:],
                        start=(dc == 0), stop=False)
                nc.tensor.matmul(ps[:], ones_row[:], bv_sb[:],
                                 start=False, stop=True)
                kc_i = sc * (QT // 128) + sub
                for p in range(2):
                    src = ps[:, ds(p * 2 * DEPTH, 2 * DEPTH)].rearrange(
                        "p (h x) -> p h x", x=DEPTH)
                    dstv = vpr[p][:, kc_i, :].rearrange(
                        "p (h x) -> p h x", x=DEPTH + 1)
                    nc.vector.tensor_copy(dstv[:, :, 0:DEPTH], src)

            # ================= P1: K + V projections =================
            for sc in range(nsc):
                xk = load_x(kT, sc, nc.sync)
                for f in range(2):
                    qkproj_emit(sc, xk, wk_sb, bk_sb, ktr, f)
                xv = load_x(vT, sc, nc.gpsimd)
                for sub in range(QT // 128):
                    vproj_emit(sc, xv, sub)
            # first q chunk up front; the rest interleaves into pair 0.
            xq = load_x(qT, 0, nc.sync)
            for f in range(2):
                qkproj_emit(0, xq, wq_sb, bq_sb, qtr, f)

            # ================= P2: attention =================
            # deferred-emission queue: small closures (qproj chunks,
            # epilogue chunks, Wo chunks) drained between slots.
            pending = []

            def drain(n=1):
                for _ in range(n):
                    if pending:
                        pending.pop(0)()

            def make_epi_chunks(pair, qt, u_sbs):
                """normalize U' -> O^T; pair0 -> ot0 resident, pair1 ->
                ot_acc tile; pair1 also appends the Wo chunks for qt."""
                chunks = []
                if pair == 1:
                    ot_acc = otp.tile([128, QT], dt, tag="ot_acc",
                                      name="ot_acc")
                else:
                    ot_acc = None
                for h in range(2):
                    u_sb = u_sbs[h]
                    for blk in range(QT // 128):
                        # phase 1: transpose + reciprocal + normalize-mul;
                        # phase 2 (a later drain round): transpose back +
                        # store. Splitting keeps the PE stream from ever
                        # waiting on a just-issued DVE/ACT op.
                        oq = epi.tile([128, DEPTH], F32, tag="oq",
                                      bufs=10, name="oq")

                        def c_n1(blk=blk, u_sb=u_sb, oq=oq):
                            t1 = ps_tile([128, DEPTH + 1], "t1")
                            nc.tensor.transpose(
                                t1[:], u_sb[:, ts(blk, 128)],
                                ident[0:DEPTH + 1, 0:DEPTH + 1])
                            rq = epi.tile([128, 1], F32, tag="rq", bufs=10)
                            nc.vector.reciprocal(
                                rq[:], t1[:, DEPTH:DEPTH + 1])
                            nc.vector.tensor_scalar_mul(
                                oq[:], t1[:, 0:DEPTH], rq[:])

                        def c_n2(h=h, blk=blk, oq=oq):
                            t2 = ps_tile([DEPTH, 128], "t2")
                            nc.tensor.transpose(t2[:], oq[:], ident[:])
                            if pair == 0:
                                dst = ot0[ds(DEPTH * h, DEPTH),
                                          ds(qt * QT + blk * 128, 128)]
                            else:
                                dst = ot_acc[ds(DEPTH * h, DEPTH),
                                             ts(blk, 128)]
                            nc.vector.tensor_copy(dst, t2[:])
                        chunks.append(c_n1)
                        chunks.append(c_n2)
                if pair == 1:
                    for qs in range(QT // 128):
                        for f in range(2):
                            def c_wo(qs=qs, f=f, ot_acc=ot_acc, qt=qt):
                                yp = ps_tile([128, 512], "yp")
                                nc.tensor.matmul(
                                    yp[:],
                                    ot0[:, ds(qt * QT + qs * 128, 128)],
                                    wo_sb[0][:, ts(f, 512)],
                                    start=True, stop=False)
                                nc.tensor.matmul(
                                    yp[:], ot_acc[:, ts(qs, 128)],
                                    wo_sb[1][:, ts(f, 512)],
                                    start=False, stop=True)
                                ysb = ypool.tile([128, 512], F32, tag="ysb",
                                                 bufs=4)
                                nc.scalar.copy(ysb[:], yp[:])
                                nc.gpsimd.dma_start(
                                    Y[ds(qt * QT + qs * 128, 128),
                                      ts(f, 512)], ysb[:])
                            chunks.append(c_wo)
                return chunks

            def make_qproj_chunks(sc):
                chunks = []
                xq = [None]

                def c_load(sc=sc):
                    xq[0] = load_x(qT, sc, nc.sync)
                chunks.append(c_load)
                for f in range(2):
                    def c_proj(sc=sc, f=f):
                        qkproj_emit(sc, xq[0], wq_sb, bq_sb, qtr, f)
                    chunks.append(c_proj)
                return chunks

            sch_b = SCH_B0 + boff

            for pair in range(2):
                ktp, qtp_, vpp = ktr[pair], qtr[pair], vpr[pair]
                for qt in range(nqt):
                    if pair == 0 and qt + 1 < nsc:
                        pending.extend(make_qproj_chunks(qt + 1))
                    pv_ps = [ps_pv.tile([DEPTH + 1, QT], F32,
                                        tag=f"pv{h}", bufs=1, name=f"pv{h}")
                             for h in range(2)]
                    lg_tiles = {}

                    def emit_lg(i, qt=qt, ktp=ktp, qtp_=qtp_,
                                lg_tiles=lg_tiles):
                        lg = ps_logit.tile([128, 2 * QT], F32, tag="lg",
                                           bufs=3, name="lg")
                        for h in range(2):
                            nc.tensor.matmul(
                                lg[:, ts(h, QT)],
                                ktp[ds(DEPTH * h, DEPTH), ts(i, KC)],
                                qtp_[ds(DEPTH * h, DEPTH), ts(qt, QT)],
                                start=True, stop=True,
                                tile_position=(DEPTH * h, 0))
                        lg_tiles[i] = lg

                    # software-pipelined slot loop: one slot = one k-chunk
                    # (both heads); exp(i), pv(i), then lg(i+3) so every
                    # PE instruction's deps are satisfied on arrival.
                    LOOK = 3
                    for i in range(min(LOOK, nkc)):
                        emit_lg(i)
                    for i in range(nkc):
                        lg = lg_tiles.pop(i)
                        use_dve = (((i + 1) * dve_num) // dve_den) != (
                            (i * dve_num) // dve_den)
                        if use_dve:
                            pt_i = ppool.tile([128, 2 * QT], I16,
                                              tag="ptD", bufs=2, name="ptD")
                            nc.vector.tensor_scalar(
                                pt_i[:], lg[:], SCH_A, sch_b,
                                op0=ALU.mult, op1=ALU.add)
                            pt = pt_i[:].bitcast(dt)
                        else:
                            pt_b = ppool.tile([128, 2 * QT], dt,
                                              tag="ptA", bufs=3, name="ptA")
                            nc.scalar.activation(
                                pt_b[:], lg[:], AFT.Exp, scale=SCALE)
                            pt = pt_b[:]
                        for h in range(2):
                            nc.tensor.matmul(
                                pv_ps[h][:],
                                vpp[:, i, ds(h * (DEPTH + 1), DEPTH + 1)],
                                pt[:, ts(h, QT)],
                                start=(i == 0), stop=(i == nkc - 1))
                        if i + LOOK < nkc:
                            emit_lg(i + LOOK)
                        if i % 2 == 1:
                            drain(1)

                    # spill U' out of PSUM (frees pv banks), defer the rest
                    u_sbs = []
                    for h in range(2):
                        u_sb = epi.tile([DEPTH + 1, QT], F32, tag="u_sb",
                                        bufs=6)
                        nc.vector.tensor_copy(u_sb[:], pv_ps[h][:])
                        u_sbs.append(u_sb)
                    pending.extend(make_epi_chunks(pair, qt, u_sbs))
            drain(len(pending))
    nc.compile()
    return nc


_NC_CACHE = {}


def _get_program(key_args):
    if key_args not in _NC_CACHE:
        _NC_CACHE[key_args] = build_program(*key_args)
    return _NC_CACHE[key_args]


def make_in_maps(inputs, seq=S):
    """Host-side sharding: per-core input dicts."""
    try:
        import ml_dtypes
        bf16 = ml_dtypes.bfloat16
    except ImportError:
        bf16 = None

    def cast(x):
        return x.astype(bf16)

    q = np.asarray(inputs["q"], np.float32)
    k = np.asarray(inputs["k"], np.float32)
    v = np.asarray(inputs["v"], np.float32)
    Wq = np.asarray(inputs["Wq"], np.float32)
    Wk = np.asarray(inputs["Wk"], np.float32)
    Wv = np.asarray(inputs["Wv"], np.float32)
    Wo = np.asarray(inputs["Wo"], np.float32)
    bq = np.asarray(inputs["bq"], np.float32)
    bk = np.asarray(inputs["bk"], np.float32)
    bv = np.asarray(inputs["bv"], np.float32)

    qTb = [np.ascontiguousarray(q[b].T) for b in range(B)]
    kTb = [np.ascontiguousarray(k[b].T) for b in range(B)]
    vTb = [np.ascontiguousarray(v[b].T) for b in range(B)]

    in_maps = []
    for c in range(8):
        b, g = c // G, c % G
        cols = slice(g * DG, (g + 1) * DG)
        in_maps.append({
            "qT": cast(qTb[b]), "kT": cast(kTb[b]), "vT": cast(vTb[b]),
            "Wq": cast(np.ascontiguousarray(Wq[:, cols])),
            "Wk": cast(np.ascontiguousarray(Wk[:, cols])),
            "Wv": cast(np.ascontiguousarray(Wv[:, cols])),
            "Wo": cast(np.ascontiguousarray(Wo[cols, :])),
            "bq": np.ascontiguousarray(bq[cols].reshape(DG, 1)),
            "bk": np.ascontiguousarray(bk[cols].reshape(DG, 1)),
            "bv": cast(np.ascontiguousarray(bv[cols].reshape(1, DG))),
        })
    return in_maps


LAST_RESULT = None


def kernel(**inputs):
    global LAST_RESULT
    dve_num = int(os.environ.get("MHA_DVE_NUM", "1"))
    dve_den = int(os.environ.get("MHA_DVE_DEN", "3"))
    boff = float(os.environ.get("MHA_BOFF", "-7.4"))
    nc = _get_program((S, dve_num, dve_den, boff))
    in_maps = make_in_maps(inputs, S)
    res = run_bass_kernel_spmd(nc, in_maps, list(range(8)))
    LAST_RESULT = res
    bo = np.asarray(inputs["bo"], np.float32)
    out = np.zeros((B, S, D), np.float32)
    for c in range(8):
        b = c // G
        out[b] += res.results[c]["Y"]
    out += bo[None, None, :]
    return out


if __name__ == "__main__":
    # smoke build
    nc = build_program(1024)
    print("built ok")


# revision 25
# speedup vs baseline: 1.0048x; 1.0048x over previous
"""Trainium2 Bass kernel for nn_MultiHeadAttention_67731634258682.

MHA: B=2, S=8192, D=1024, H=16 heads (depth 64).
Sharding over 8 cores: core c -> (batch b = c//4, head-group g = c%4).
Each core computes its 4 heads end-to-end plus a row-parallel partial of
the output projection; the host sums the 4 partials per batch.

v2 design (vs v1 baseline at 3.09 ms):
  - Everything SBUF-resident: K^T/Q^T (2 pairs x [128, S]), V' (2 pairs x
    [128, nkc, 130] with ones columns), pair-0 O^T [128, S]. Projections
    evacuate straight into the resident tiles (no DRAM scratch round-trip).
  - Exp split across engines: ~2/3 of (kc) slots on ScalarE (exact Exp,
    scale folded), ~1/3 on DVE via a Schraudolph bit-trick: bf16 bits =
    int16(round(A*logit + B)), one fused tensor_scalar per slot. Rel-err
    cost measured in simulation: 6.0e-3 -> 1.2e-2 (gate 2e-2).
  - Software-pipelined emission: per slot emit exp(i), lg(i+2), pv(i) so
    the in-order PE stream never head-of-line blocks on an exp; epilogue /
    Wo / next-tile q-projection are chopped into small chunks emitted
    between slots to keep the PE continuously busy (p-state ramp to 2.4
    GHz requires gapless execution).
  - DMA issue moved off ScalarE entirely (sync for loads, gpsimd for Y).
"""

import os
import sys
import numpy as np

for _p in ("/opt/trn_rl_repo", "/root/.axon_site/_ro/trn_rl_repo"):
    if os.path.isdir(_p) and _p not in sys.path:
        sys.path.append(_p)

import concourse.bass as bass
import concourse.mybir as mybir
from concourse import bacc, tile
from concourse.bass import ts, ds
from concourse.masks import make_identity
from concourse.bass_utils import run_bass_kernel_spmd

F32 = mybir.dt.float32
BF16 = mybir.dt.bfloat16
I16 = mybir.dt.int16

B, S, D = 2, 8192, 1024
H = 16
DEPTH = 64          # head dim
G = 4               # head groups (one per core within a batch)
HPG = 4             # heads per group
DG = HPG * DEPTH    # 256 features per group
QT = 512            # q tile
KC = 128            # k chunk (matmul contraction tile)
NDC = D // 128      # 8 contraction chunks for projections

AFT = mybir.ActivationFunctionType
ALU = mybir.AluOpType

SCALE = 0.125                                  # 1/sqrt(64)
SCH_A = SCALE * np.log2(np.e) * 128.0          # schraudolph multiplier
SCH_B0 = 127.0 * 128.0                         # exponent bias in bf16 bits


def build_program(seq=S, dve_num=1, dve_den=3, boff=-7.4):
    """Build the per-core Bass program. Returns the compiled Bacc object."""
    assert seq % QT == 0
    nqt = seq // QT
    nkc = seq // KC
    nsc = seq // QT
    dt = BF16

    nc = bacc.Bacc("TRN2", target_bir_lowering=False, debug=False,
                   enable_asserts=False, num_devices=8)

    # ---- external I/O ----
    qT = nc.dram_tensor("qT", [D, seq], dt, kind="ExternalInput").ap()
    kT = nc.dram_tensor("kT", [D, seq], dt, kind="ExternalInput").ap()
    vT = nc.dram_tensor("vT", [D, seq], dt, kind="ExternalInput").ap()
    Wq = nc.dram_tensor("Wq", [D, DG], dt, kind="ExternalInput").ap()
    Wk = nc.dram_tensor("Wk", [D, DG], dt, kind="ExternalInput").ap()
    Wv = nc.dram_tensor("Wv", [D, DG], dt, kind="ExternalInput").ap()
    Wo = nc.dram_tensor("Wo", [DG, D], dt, kind="ExternalInput").ap()
    bq = nc.dram_tensor("bq", [DG, 1], F32, kind="ExternalInput").ap()
    bk = nc.dram_tensor("bk", [DG, 1], F32, kind="ExternalInput").ap()
    bv = nc.dram_tensor("bv", [1, DG], dt, kind="ExternalInput").ap()
    Y = nc.dram_tensor("Y", [seq, D], F32, kind="ExternalOutput").ap()

    with tile.TileContext(nc) as tc:
        from contextlib import ExitStack
        ctx = ExitStack()
        with ctx:
            const = ctx.enter_context(tc.tile_pool(name="const", bufs=1))
            res = ctx.enter_context(tc.tile_pool(name="res", bufs=1))
            xin = ctx.enter_context(tc.tile_pool(name="xin", bufs=3))
            ppool = ctx.enter_context(tc.tile_pool(name="ppool", bufs=3))
            epi = ctx.enter_context(tc.tile_pool(name="epi", bufs=4))
            otp = ctx.enter_context(tc.tile_pool(name="otp", bufs=3))
            ypool = ctx.enter_context(tc.tile_pool(name="ypool", bufs=3))
            # One shared PSUM ring: 3 slots of 2 banks each (tag "lg") serve
            # the logits tiles AND all small PE outputs (transposes, Wo
            # accumulator, projection accumulator); pv pins the last 2 banks.
            ps_logit = ctx.enter_context(
                tc.tile_pool(name="ps_logit", bufs=3, space="PSUM"))
            ps_pv = ctx.enter_context(
                tc.tile_pool(name="ps_pv", bufs=1, space="PSUM"))

            def ps_tile(shape, name):
                return ps_logit.tile(shape, F32, tag="lg", bufs=3, name=name)

            # ---- constants ----
            ident = const.tile([128, 128], F32, tag="ident")
            make_identity(nc, ident[:])
            ones_f32 = const.tile([128, 128], F32, tag="ones_f32")
            nc.any.memset(ones_f32[:], 1.0)
            ones_row = const.tile([1, 128], dt, tag="ones_row")
            nc.vector.tensor_copy(ones_row[:], ones_f32[0:1, :])
            ident_bf = const.tile([128, 128], dt, tag="ident_bf")
            nc.vector.tensor_copy(ident_bf[:], ident[:])

            wq_sb = [const.tile([128, DG], dt, tag=f"wq{dc}", name=f"wq{dc}")
                     for dc in range(NDC)]
            wk_sb = [const.tile([128, DG], dt, tag=f"wk{dc}", name=f"wk{dc}")
                     for dc in range(NDC)]
            wv_sb = [const.tile([128, DG], dt, tag=f"wv{dc}", name=f"wv{dc}")
                     for dc in range(NDC)]
            for dc in range(NDC):
                nc.sync.dma_start(wq_sb[dc][:], Wq[ts(dc, 128), :])
                nc.sync.dma_start(wk_sb[dc][:], Wk[ts(dc, 128), :])
                nc.sync.dma_start(wv_sb[dc][:], Wv[ts(dc, 128), :])
            wo_sb = [const.tile([128, D], dt, tag=f"wo{i}", name=f"wo{i}")
                     for i in range(2)]
            for i in range(2):
                nc.sync.dma_start(wo_sb[i][:], Wo[ts(i, 128), :])
            bq_sb = [const.tile([128, 1], F32, tag=f"bq{i}", name=f"bq{i}")
                     for i in range(2)]
            bk_sb = [const.tile([128, 1], F32, tag=f"bk{i}", name=f"bk{i}")
                     for i in range(2)]
            for i in range(2):
                nc.sync.dma_start(bq_sb[i][:], bq[ts(i, 128), :])
                nc.sync.dma_start(bk_sb[i][:], bk[ts(i, 128), :])
            bv_sb = const.tile([1, DG], dt, tag="bv_sb")
            nc.sync.dma_start(bv_sb[:], bv[:, :])

            # ---- resident tensors (persist for the whole kernel) ----
            ktr = [res.tile([128, seq], dt, tag=f"ktr{p}", name=f"ktr{p}")
                   for p in range(2)]
            qtr = [res.tile([128, seq], dt, tag=f"qtr{p}", name=f"qtr{p}")
                   for p in range(2)]
            VP_W = 2 * (DEPTH + 1)  # per-pair per-chunk: 2 heads x [V_h|1]
            vpr = [res.tile([128, nkc, VP_W], dt, tag=f"vpr{p}",
                            name=f"vpr{p}") for p in range(2)]
            ot0 = res.tile([128, seq], dt, tag="ot0", name="ot0")
            # ones columns of V' are constant: write once.
            for p in range(2):
                vh = vpr[p][:].rearrange("p k (h x) -> p k h x", x=DEPTH + 1)
                nc.vector.memset(vh[:, :, :, DEPTH:DEPTH + 1], 1.0)

            # ---- projection emitters ----
            def load_x(src, sc, eng):
                xt = xin.tile([128, NDC, QT], dt, tag="xt", name="xt")
                rr = src.rearrange("(c p) s -> p c s", p=128)
                eng.dma_start(xt[:], rr[:, :, ts(sc, QT)])
                return xt

            def qkproj_emit(sc, xt, w_sb, b_sb, dst, f, on_act=False):
                ps = ps_tile([128, QT], "ps")
                for dc in range(NDC):
                    nc.tensor.matmul(
                        ps[:], w_sb[dc][:, ts(f, 128)], xt[:, dc, :],
                        start=(dc == 0), stop=(dc == NDC - 1))
                if on_act:
                    # P1 runs before attention: ScalarE is idle there, so
                    # evacuate on it to keep DVE free.
                    nc.scalar.activation(
                        dst[f][:, ts(sc, QT)], ps[:], AFT.Identity,
                        bias=b_sb[f][:])
                else:
                    nc.vector.tensor_scalar_add(
                        dst[f][:, ts(sc, QT)], ps[:], b_sb[f][:])

            def vproj_emit(sc, xt, sub):
                ps = ps_tile([128, DG], "ps")
                for dc in range(NDC):
                    nc.tensor.matmul(
                        ps[:], xt[:, dc, ts(sub, 128)], wv_sb[dc][:],
                        start=(dc == 0), stop=False)
                nc.tensor.matmul(ps[:], ones_row[:], bv_sb[:],
                                 start=False, stop=True)
                kc_i = sc * (QT // 128) + sub
                for p in range(2):
                    src = ps[:, ds(p * 2 * DEPTH, 2 * DEPTH)].rearrange(
                        "p (h x) -> p h x", x=DEPTH)
                    dstv = vpr[p][:, kc_i, :].rearrange(
                        "p (h x) -> p h x", x=DEPTH + 1)
                    nc.scalar.copy(dstv[:, :, 0:DEPTH], src)

            # ================= P1: K + V projections =================
            for sc in range(nsc):
                xk = load_x(kT, sc, nc.sync)
                for f in range(2):
                    qkproj_emit(sc, xk, wk_sb, bk_sb, ktr, f, on_act=True)
                xv = load_x(vT, sc, nc.gpsimd)
                for sub in range(QT // 128):
                    vproj_emit(sc, xv, sub)
            # first q chunk up front; the rest interleaves into pair 0.
            xq = load_x(qT, 0, nc.sync)
            for f in range(2):
                qkproj_emit(0, xq, wq_sb, bq_sb, qtr, f, on_act=True)

            # ================= P2: attention =================
            # deferred-emission queue: small closures (qproj chunks,
            # epilogue chunks, Wo chunks) drained between slots.
            pending = []

            def drain(n=1):
                for _ in range(n):
                    if pending:
                        pending.pop(0)()

            def make_epi_chunks(pair, qt, u_sbs):
                """normalize U' -> O^T; pair0 -> ot0 resident, pair1 ->
                ot_acc tile; pair1 also appends the Wo chunks for qt."""
                chunks = []
                if pair == 1:
                    ot_acc = otp.tile([128, QT], dt, tag="ot_acc",
                                      name="ot_acc")
                else:
                    ot_acc = None
                # phase 1 chunks (all emitted before any phase 2): transpose
                # + reciprocal + normalize-mul; phase 2: transpose back +
                # store. Separating the phases by many drain rounds keeps
                # the PE stream from ever waiting on a just-issued DVE op.
                phase2 = []
                for h in range(2):
                    u_sb = u_sbs[h]
                    for blk in range(QT // 128):
                        oq = epi.tile([128, DEPTH], F32, tag="oq",
                                      bufs=10, name="oq")

                        def c_n1(blk=blk, u_sb=u_sb, oq=oq):
                            t1 = ps_tile([128, DEPTH + 1], "t1")
                            nc.tensor.transpose(
                                t1[:], u_sb[:, ts(blk, 128)],
                                ident[0:DEPTH + 1, 0:DEPTH + 1])
                            rq = epi.tile([128, 1], F32, tag="rq", bufs=10)
                            nc.vector.reciprocal(
                                rq[:], t1[:, DEPTH:DEPTH + 1])
                            nc.vector.tensor_scalar_mul(
                                oq[:], t1[:, 0:DEPTH], rq[:])

                        def c_n2(h=h, blk=blk, oq=oq):
                            t2 = ps_tile([DEPTH, 128], "t2")
                            nc.tensor.transpose(t2[:], oq[:], ident[:])
                            if pair == 0:
                                dst = ot0[ds(DEPTH * h, DEPTH),
                                          ds(qt * QT + blk * 128, 128)]
                            else:
                                dst = ot_acc[ds(DEPTH * h, DEPTH),
                                             ts(blk, 128)]
                            nc.vector.tensor_copy(dst, t2[:])
                        chunks.append(c_n1)
                        phase2.append(c_n2)
                chunks.extend(phase2)
                if pair == 1:
                    for qs in range(QT // 128):
                        def c_wo(qs=qs, ot_acc=ot_acc, qt=qt):
                            for f in range(2):
                                yp = ps_tile([128, 512], "yp")
                                nc.tensor.matmul(
                                    yp[:],
                                    ot0[:, ds(qt * QT + qs * 128, 128)],
                                    wo_sb[0][:, ts(f, 512)],
                                    start=True, stop=False)
                                nc.tensor.matmul(
                                    yp[:], ot_acc[:, ts(qs, 128)],
                                    wo_sb[1][:, ts(f, 512)],
                                    start=False, stop=True)
                                ysb = ypool.tile([128, 512], F32, tag="ysb",
                                                 bufs=4)
                                nc.scalar.copy(ysb[:], yp[:])
                                nc.gpsimd.dma_start(
                                    Y[ds(qt * QT + qs * 128, 128),
                                      ts(f, 512)], ysb[:])
                        chunks.append(c_wo)
                return chunks

            def make_qproj_chunks(sc):
                chunks = []
                xq = [None]

                def c_load(sc=sc):
                    xq[0] = load_x(qT, sc, nc.sync)
                chunks.append(c_load)
                for f in range(2):
                    def c_proj(sc=sc, f=f):
                        qkproj_emit(sc, xq[0], wq_sb, bq_sb, qtr, f)
                    chunks.append(c_proj)
                return chunks

            sch_b = SCH_B0 + boff

            for pair in range(2):
                ktp, qtp_, vpp = ktr[pair], qtr[pair], vpr[pair]
                for qt in range(nqt):
                    if pair == 0 and qt + 1 < nsc:
                        pending.extend(make_qproj_chunks(qt + 1))
                    pv_ps = [ps_pv.tile([DEPTH + 1, QT], F32,
                                        tag=f"pv{h}", bufs=1, name=f"pv{h}")
                             for h in range(2)]
                    lg_tiles = {}

                    def emit_lg(i, qt=qt, ktp=ktp, qtp_=qtp_,
                                lg_tiles=lg_tiles):
                        lg = ps_logit.tile([128, 2 * QT], F32, tag="lg",
                                           bufs=3, name="lg")
                        for h in range(2):
                            nc.tensor.matmul(
                                lg[:, ts(h, QT)],
                                ktp[ds(DEPTH * h, DEPTH), ts(i, KC)],
                                qtp_[ds(DEPTH * h, DEPTH), ts(qt, QT)],
                                start=True, stop=True,
                                tile_position=(DEPTH * h, 0))
                        lg_tiles[i] = lg

                    # software-pipelined slot loop: one slot = one k-chunk
                    # (both heads); exp(i), pv(i), then lg(i+3) so every
                    # PE instruction's deps are satisfied on arrival.
                    LOOK = 3
                    for i in range(min(LOOK, nkc)):
                        emit_lg(i)
                    for i in range(nkc):
                        lg = lg_tiles.pop(i)
                        use_dve = (((i + 1) * dve_num) // dve_den) != (
                            (i * dve_num) // dve_den)
                        if use_dve:
                            pt_i = ppool.tile([128, 2 * QT], I16,
                                              tag="ptD", bufs=2, name="ptD")
                            nc.vector.tensor_scalar(
                                pt_i[:], lg[:], SCH_A, sch_b,
                                op0=ALU.mult, op1=ALU.add)
                            pt = pt_i[:].bitcast(dt)
                        else:
                            pt_b = ppool.tile([128, 2 * QT], dt,
                                              tag="ptA", bufs=3, name="ptA")
                            nc.scalar.activation(
                                pt_b[:], lg[:], AFT.Exp, scale=SCALE)
                            pt = pt_b[:]
                        for h in range(2):
                            nc.tensor.matmul(
                                pv_ps[h][:],
                                vpp[:, i, ds(h * (DEPTH + 1), DEPTH + 1)],
                                pt[:, ts(h, QT)],
                                start=(i == 0), stop=(i == nkc - 1))
                        if i + LOOK < nkc:
                            emit_lg(i + LOOK)
                        if i % 3 == 2:
                            drain(1)

                    # spill U' out of PSUM (frees pv banks), defer the rest
                    u_sbs = []
                    for h in range(2):
                        u_sb = epi.tile([DEPTH + 1, QT], F32, tag="u_sb",
                                        bufs=6)
                        nc.vector.tensor_copy(u_sb[:], pv_ps[h][:])
                        u_sbs.append(u_sb)
                    pending.extend(make_epi_chunks(pair, qt, u_sbs))
            drain(len(pending))
    nc.compile()
    return nc


_NC_CACHE = {}


def _get_program(key_args):
    if key_args not in _NC_CACHE:
        _NC_CACHE[key_args] = build_program(*key_args)
    return _NC_CACHE[key_args]


def make_in_maps(inputs, seq=S):
    """Host-side sharding: per-core input dicts."""
    try:
        import ml_dtypes
        bf16 = ml_dtypes.bfloat16
    except ImportError:
        bf16 = None

    def cast(x):
        return x.astype(bf16)

    q = np.asarray(inputs["q"], np.float32)
    k = np.asarray(inputs["k"], np.float32)
    v = np.asarray(inputs["v"], np.float32)
    Wq = np.asarray(inputs["Wq"], np.float32)
    Wk = np.asarray(inputs["Wk"], np.float32)
    Wv = np.asarray(inputs["Wv"], np.float32)
    Wo = np.asarray(inputs["Wo"], np.float32)
    bq = np.asarray(inputs["bq"], np.float32)
    bk = np.asarray(inputs["bk"], np.float32)
    bv = np.asarray(inputs["bv"], np.float32)

    qTb = [np.ascontiguousarray(q[b].T) for b in range(B)]
    kTb = [np.ascontiguousarray(k[b].T) for b in range(B)]
    vTb = [np.ascontiguousarray(v[b].T) for b in range(B)]

    in_maps = []
    for c in range(8):
        b, g = c // G, c % G
        cols = slice(g * DG, (g + 1) * DG)
        in_maps.append({
            "qT": cast(qTb[b]), "kT": cast(kTb[b]), "vT": cast(vTb[b]),
            "Wq": cast(np.ascontiguousarray(Wq[:, cols])),
            "Wk": cast(np.ascontiguousarray(Wk[:, cols])),
            "Wv": cast(np.ascontiguousarray(Wv[:, cols])),
            "Wo": cast(np.ascontiguousarray(Wo[cols, :])),
            "bq": np.ascontiguousarray(bq[cols].reshape(DG, 1)),
            "bk": np.ascontiguousarray(bk[cols].reshape(DG, 1)),
            "bv": cast(np.ascontiguousarray(bv[cols].reshape(1, DG))),
        })
    return in_maps


LAST_RESULT = None


def kernel(**inputs):
    global LAST_RESULT
    dve_num = int(os.environ.get("MHA_DVE_NUM", "2"))
    dve_den = int(os.environ.get("MHA_DVE_DEN", "5"))
    boff = float(os.environ.get("MHA_BOFF", "-7.4"))
    nc = _get_program((S, dve_num, dve_den, boff))
    in_maps = make_in_maps(inputs, S)
    res = run_bass_kernel_spmd(nc, in_maps, list(range(8)))
    LAST_RESULT = res
    bo = np.asarray(inputs["bo"], np.float32)
    out = np.zeros((B, S, D), np.float32)
    for c in range(8):
        b = c // G
        out[b] += res.results[c]["Y"]
    out += bo[None, None, :]
    return out


if __name__ == "__main__":
    # smoke build
    nc = build_program(1024)
    print("built ok")


# revision 26
# speedup vs baseline: 1.0292x; 1.0243x over previous
"""Trainium2 Bass kernel for nn_MultiHeadAttention_67731634258682.

MHA: B=2, S=8192, D=1024, H=16 heads (depth 64).
Sharding over 8 cores: core c -> (batch b = c//4, head-group g = c%4).
Each core computes its 4 heads end-to-end plus a row-parallel partial of
the output projection; the host sums the 4 partials per batch.

v2 design (vs v1 baseline at 3.09 ms):
  - Everything SBUF-resident: K^T/Q^T (2 pairs x [128, S]), V' (2 pairs x
    [128, nkc, 130] with ones columns), pair-0 O^T [128, S]. Projections
    evacuate straight into the resident tiles (no DRAM scratch round-trip).
  - Exp split across engines: ~2/3 of (kc) slots on ScalarE (exact Exp,
    scale folded), ~1/3 on DVE via a Schraudolph bit-trick: bf16 bits =
    int16(round(A*logit + B)), one fused tensor_scalar per slot. Rel-err
    cost measured in simulation: 6.0e-3 -> 1.2e-2 (gate 2e-2).
  - Software-pipelined emission: per slot emit exp(i), lg(i+2), pv(i) so
    the in-order PE stream never head-of-line blocks on an exp; epilogue /
    Wo / next-tile q-projection are chopped into small chunks emitted
    between slots to keep the PE continuously busy (p-state ramp to 2.4
    GHz requires gapless execution).
  - DMA issue moved off ScalarE entirely (sync for loads, gpsimd for Y).
"""

import os
import sys
import numpy as np

for _p in ("/opt/trn_rl_repo", "/root/.axon_site/_ro/trn_rl_repo"):
    if os.path.isdir(_p) and _p not in sys.path:
        sys.path.append(_p)

import concourse.bass as bass
import concourse.mybir as mybir
from concourse import bacc, tile
from concourse.bass import ts, ds
from concourse.masks import make_identity
from concourse.bass_utils import run_bass_kernel_spmd

F32 = mybir.dt.float32
BF16 = mybir.dt.bfloat16
I16 = mybir.dt.int16

B, S, D = 2, 8192, 1024
H = 16
DEPTH = 64          # head dim
G = 4               # head groups (one per core within a batch)
HPG = 4             # heads per group
DG = HPG * DEPTH    # 256 features per group
QT = 512            # q tile
KC = 128            # k chunk (matmul contraction tile)
NDC = D // 128      # 8 contraction chunks for projections

AFT = mybir.ActivationFunctionType
ALU = mybir.AluOpType

SCALE = 0.125                                  # 1/sqrt(64)
SCH_A = SCALE * np.log2(np.e) * 128.0          # schraudolph multiplier
SCH_B0 = 127.0 * 128.0                         # exponent bias in bf16 bits


def build_program(seq=S, dve_num=1, dve_den=3, boff=-7.4):
    """Build the per-core Bass program. Returns the compiled Bacc object."""
    assert seq % QT == 0
    nqt = seq // QT
    nkc = seq // KC
    nsc = seq // QT
    dt = BF16

    nc = bacc.Bacc("TRN2", target_bir_lowering=False, debug=False,
                   enable_asserts=False, num_devices=8)

    # ---- external I/O ----
    qT = nc.dram_tensor("qT", [D, seq], dt, kind="ExternalInput").ap()
    kT = nc.dram_tensor("kT", [D, seq], dt, kind="ExternalInput").ap()
    vT = nc.dram_tensor("vT", [D, seq], dt, kind="ExternalInput").ap()
    Wq = nc.dram_tensor("Wq", [D, DG], dt, kind="ExternalInput").ap()
    Wk = nc.dram_tensor("Wk", [D, DG], dt, kind="ExternalInput").ap()
    Wv = nc.dram_tensor("Wv", [D, DG], dt, kind="ExternalInput").ap()
    Wo = nc.dram_tensor("Wo", [DG, D], dt, kind="ExternalInput").ap()
    bq = nc.dram_tensor("bq", [DG, 1], F32, kind="ExternalInput").ap()
    bk = nc.dram_tensor("bk", [DG, 1], F32, kind="ExternalInput").ap()
    bv = nc.dram_tensor("bv", [1, DG], dt, kind="ExternalInput").ap()
    Y = nc.dram_tensor("Y", [seq, D], F32, kind="ExternalOutput").ap()

    with tile.TileContext(nc) as tc:
        from contextlib import ExitStack
        ctx = ExitStack()
        with ctx:
            const = ctx.enter_context(tc.tile_pool(name="const", bufs=1))
            res = ctx.enter_context(tc.tile_pool(name="res", bufs=1))
            xin = ctx.enter_context(tc.tile_pool(name="xin", bufs=3))
            ppool = ctx.enter_context(tc.tile_pool(name="ppool", bufs=3))
            epi = ctx.enter_context(tc.tile_pool(name="epi", bufs=4))
            otp = ctx.enter_context(tc.tile_pool(name="otp", bufs=3))
            ypool = ctx.enter_context(tc.tile_pool(name="ypool", bufs=3))
            # One shared PSUM ring: 3 slots of 2 banks each (tag "lg") serve
            # the logits tiles AND all small PE outputs (transposes, Wo
            # accumulator, projection accumulator); pv pins the last 2 banks.
            ps_logit = ctx.enter_context(
                tc.tile_pool(name="ps_logit", bufs=3, space="PSUM"))
            ps_pv = ctx.enter_context(
                tc.tile_pool(name="ps_pv", bufs=1, space="PSUM"))

            def ps_tile(shape, name):
                return ps_logit.tile(shape, F32, tag="lg", bufs=3, name=name)

            # ---- constants ----
            ident = const.tile([128, 128], F32, tag="ident")
            make_identity(nc, ident[:])
            ones_f32 = const.tile([128, 128], F32, tag="ones_f32")
            nc.any.memset(ones_f32[:], 1.0)
            ones_row = const.tile([1, 128], dt, tag="ones_row")
            nc.vector.tensor_copy(ones_row[:], ones_f32[0:1, :])
            ident_bf = const.tile([128, 128], dt, tag="ident_bf")
            nc.vector.tensor_copy(ident_bf[:], ident[:])

            wq_sb = [const.tile([128, DG], dt, tag=f"wq{dc}", name=f"wq{dc}")
                     for dc in range(NDC)]
            wk_sb = [const.tile([128, DG], dt, tag=f"wk{dc}", name=f"wk{dc}")
                     for dc in range(NDC)]
            wv_sb = [const.tile([128, DG], dt, tag=f"wv{dc}", name=f"wv{dc}")
                     for dc in range(NDC)]
            for dc in range(NDC):
                nc.sync.dma_start(wq_sb[dc][:], Wq[ts(dc, 128), :])
                nc.sync.dma_start(wk_sb[dc][:], Wk[ts(dc, 128), :])
                nc.sync.dma_start(wv_sb[dc][:], Wv[ts(dc, 128), :])
            wo_sb = [const.tile([128, D], dt, tag=f"wo{i}", name=f"wo{i}")
                     for i in range(2)]
            for i in range(2):
                nc.sync.dma_start(wo_sb[i][:], Wo[ts(i, 128), :])
            bq_sb = [const.tile([128, 1], F32, tag=f"bq{i}", name=f"bq{i}")
                     for i in range(2)]
            bk_sb = [const.tile([128, 1], F32, tag=f"bk{i}", name=f"bk{i}")
                     for i in range(2)]
            for i in range(2):
                nc.sync.dma_start(bq_sb[i][:], bq[ts(i, 128), :])
                nc.sync.dma_start(bk_sb[i][:], bk[ts(i, 128), :])
            bv_sb = const.tile([1, DG], dt, tag="bv_sb")
            nc.sync.dma_start(bv_sb[:], bv[:, :])

            # ---- resident tensors (persist for the whole kernel) ----
            ktr = [res.tile([128, seq], dt, tag=f"ktr{p}", name=f"ktr{p}")
                   for p in range(2)]
            qtr = [res.tile([128, seq], dt, tag=f"qtr{p}", name=f"qtr{p}")
                   for p in range(2)]
            VP_W = 2 * (DEPTH + 1)  # per-pair per-chunk: 2 heads x [V_h|1]
            vpr = [res.tile([128, nkc, VP_W], dt, tag=f"vpr{p}",
                            name=f"vpr{p}") for p in range(2)]
            ot0 = res.tile([128, seq], dt, tag="ot0", name="ot0")
            # ones columns of V' are constant: write once.
            for p in range(2):
                vh = vpr[p][:].rearrange("p k (h x) -> p k h x", x=DEPTH + 1)
                nc.vector.memset(vh[:, :, :, DEPTH:DEPTH + 1], 1.0)

            # ---- projection emitters ----
            def load_x(src, sc, eng):
                xt = xin.tile([128, NDC, QT], dt, tag="xt", name="xt")
                rr = src.rearrange("(c p) s -> p c s", p=128)
                eng.dma_start(xt[:], rr[:, :, ts(sc, QT)])
                return xt

            def qkproj_emit(sc, xt, w_sb, b_sb, dst, f, on_act=False):
                ps = ps_tile([128, QT], "ps")
                for dc in range(NDC):
                    nc.tensor.matmul(
                        ps[:], w_sb[dc][:, ts(f, 128)], xt[:, dc, :],
                        start=(dc == 0), stop=(dc == NDC - 1))
                if on_act:
                    # P1 runs before attention: ScalarE is idle there, so
                    # evacuate on it to keep DVE free.
                    nc.scalar.activation(
                        dst[f][:, ts(sc, QT)], ps[:], AFT.Identity,
                        bias=b_sb[f][:])
                else:
                    nc.vector.tensor_scalar_add(
                        dst[f][:, ts(sc, QT)], ps[:], b_sb[f][:])

            def vproj_emit(sc, xt, sub):
                ps = ps_tile([128, DG], "ps")
                for dc in range(NDC):
                    nc.tensor.matmul(
                        ps[:], xt[:, dc, ts(sub, 128)], wv_sb[dc][:],
                        start=(dc == 0), stop=False)
                nc.tensor.matmul(ps[:], ones_row[:], bv_sb[:],
                                 start=False, stop=True)
                kc_i = sc * (QT // 128) + sub
                for p in range(2):
                    src = ps[:, ds(p * 2 * DEPTH, 2 * DEPTH)].rearrange(
                        "p (h x) -> p h x", x=DEPTH)
                    dstv = vpr[p][:, kc_i, :].rearrange(
                        "p (h x) -> p h x", x=DEPTH + 1)
                    nc.scalar.copy(dstv[:, :, 0:DEPTH], src)

            # ================= P1: K + V projections =================
            for sc in range(nsc):
                xk = load_x(kT, sc, nc.sync)
                for f in range(2):
                    qkproj_emit(sc, xk, wk_sb, bk_sb, ktr, f, on_act=True)
                xv = load_x(vT, sc, nc.gpsimd)
                for sub in range(QT // 128):
                    vproj_emit(sc, xv, sub)
            # first q chunk up front; the rest interleaves into pair 0.
            xq = load_x(qT, 0, nc.sync)
            for f in range(2):
                qkproj_emit(0, xq, wq_sb, bq_sb, qtr, f, on_act=True)

            # ================= P2: attention =================
            # deferred-emission queue: small closures (qproj chunks,
            # epilogue chunks, Wo chunks) drained between slots.
            pending = []

            def drain(n=1):
                for _ in range(n):
                    if pending:
                        pending.pop(0)()

            def make_epi_chunks(pair, qt, u_sbs):
                """normalize U' -> O^T; pair0 -> ot0 resident, pair1 ->
                ot_acc tile; pair1 also appends the Wo chunks for qt."""
                chunks = []
                if pair == 1:
                    ot_acc = otp.tile([128, QT], dt, tag="ot_acc",
                                      name="ot_acc")
                else:
                    ot_acc = None
                # phase 1 chunks (all emitted before any phase 2): transpose
                # + reciprocal + normalize-mul; phase 2: transpose back +
                # store. Separating the phases by many drain rounds keeps
                # the PE stream from ever waiting on a just-issued DVE op.
                phase2 = []
                for h in range(2):
                    u_sb = u_sbs[h]
                    for blk in range(QT // 128):
                        oq = epi.tile([128, DEPTH], F32, tag="oq",
                                      bufs=10, name="oq")

                        def c_n1(blk=blk, u_sb=u_sb, oq=oq):
                            t1 = ps_tile([128, DEPTH + 1], "t1")
                            nc.tensor.transpose(
                                t1[:], u_sb[:, ts(blk, 128)],
                                ident[0:DEPTH + 1, 0:DEPTH + 1])
                            rq = epi.tile([128, 1], F32, tag="rq", bufs=10)
                            nc.vector.reciprocal(
                                rq[:], t1[:, DEPTH:DEPTH + 1])
                            nc.vector.tensor_scalar_mul(
                                oq[:], t1[:, 0:DEPTH], rq[:])

                        def c_n2(h=h, blk=blk, oq=oq):
                            t2 = ps_tile([DEPTH, 128], "t2")
                            nc.tensor.transpose(t2[:], oq[:], ident[:])
                            if pair == 0:
                                dst = ot0[ds(DEPTH * h, DEPTH),
                                          ds(qt * QT + blk * 128, 128)]
                            else:
                                dst = ot_acc[ds(DEPTH * h, DEPTH),
                                             ts(blk, 128)]
                            nc.vector.tensor_copy(dst, t2[:])
                        chunks.append(c_n1)
                        phase2.append(c_n2)
                chunks.extend(phase2)
                if pair == 1:
                    for qs in range(QT // 128):
                        def c_wo(qs=qs, ot_acc=ot_acc, qt=qt):
                            for f in range(2):
                                yp = ps_tile([128, 512], "yp")
                                nc.tensor.matmul(
                                    yp[:],
                                    ot0[:, ds(qt * QT + qs * 128, 128)],
                                    wo_sb[0][:, ts(f, 512)],
                                    start=True, stop=False)
                                nc.tensor.matmul(
                                    yp[:], ot_acc[:, ts(qs, 128)],
                                    wo_sb[1][:, ts(f, 512)],
                                    start=False, stop=True)
                                ysb = ypool.tile([128, 512], F32, tag="ysb",
                                                 bufs=4)
                                nc.scalar.copy(ysb[:], yp[:])
                                nc.gpsimd.dma_start(
                                    Y[ds(qt * QT + qs * 128, 128),
                                      ts(f, 512)], ysb[:])
                        chunks.append(c_wo)
                return chunks

            def make_qproj_chunks(sc):
                chunks = []
                xq = [None]

                def c_load(sc=sc):
                    xq[0] = load_x(qT, sc, nc.sync)
                chunks.append(c_load)
                for f in range(2):
                    def c_proj(sc=sc, f=f):
                        qkproj_emit(sc, xq[0], wq_sb, bq_sb, qtr, f)
                    chunks.append(c_proj)
                return chunks

            sch_b = SCH_B0 + boff

            for pair in range(2):
                ktp, qtp_, vpp = ktr[pair], qtr[pair], vpr[pair]
                for qt in range(nqt):
                    if pair == 0 and qt + 1 < nsc:
                        pending.extend(make_qproj_chunks(qt + 1))
                    pv_ps = [ps_pv.tile([DEPTH + 1, QT], F32,
                                        tag=f"pv{h}", bufs=1, name=f"pv{h}")
                             for h in range(2)]
                    lg_tiles = {}

                    def emit_lg(i, qt=qt, ktp=ktp, qtp_=qtp_,
                                lg_tiles=lg_tiles):
                        lg = ps_logit.tile([128, 2 * QT], F32, tag="lg",
                                           bufs=3, name="lg")
                        for h in range(2):
                            nc.tensor.matmul(
                                lg[:, ts(h, QT)],
                                ktp[ds(DEPTH * h, DEPTH), ts(i, KC)],
                                qtp_[ds(DEPTH * h, DEPTH), ts(qt, QT)],
                                start=True, stop=True,
                                tile_position=(DEPTH * h, 0))
                        lg_tiles[i] = lg

                    # software-pipelined loop over BLOCKS of 2 k-chunks.
                    # Per block: exp(j) x2, pv(j) x4, lg(j+LOOK) x2. The
                    # pv-before-lg order gives every matmul exactly one
                    # pending weight-load (the logit pair's two LDWs no
                    # longer sit right before a PV), and the per-block sem
                    # waits amortize over 6 matmul streams.
                    LOOK = 4
                    for i in range(min(LOOK, nkc)):
                        emit_lg(i)
                    for b0 in range(0, nkc, 2):
                        blk = [j for j in (b0, b0 + 1) if j < nkc]
                        pts = {}
                        for j in blk:
                            lg = lg_tiles.pop(j)
                            use_dve = (((j + 1) * dve_num) // dve_den) != (
                                (j * dve_num) // dve_den)
                            if use_dve:
                                pt_i = ppool.tile([128, 2 * QT], I16,
                                                  tag="ptD", bufs=2,
                                                  name="ptD")
                                nc.vector.tensor_scalar(
                                    pt_i[:], lg[:], SCH_A, sch_b,
                                    op0=ALU.mult, op1=ALU.add)
                                pts[j] = pt_i[:].bitcast(dt)
                            else:
                                pt_b = ppool.tile([128, 2 * QT], dt,
                                                  tag="ptA", bufs=3,
                                                  name="ptA")
                                nc.scalar.activation(
                                    pt_b[:], lg[:], AFT.Exp, scale=SCALE)
                                pts[j] = pt_b[:]
                        for j in blk:
                            for h in range(2):
                                nc.tensor.matmul(
                                    pv_ps[h][:],
                                    vpp[:, j,
                                        ds(h * (DEPTH + 1), DEPTH + 1)],
                                    pts[j][:, ts(h, QT)],
                                    start=(j == 0), stop=(j == nkc - 1))
                        for j in blk:
                            if j + LOOK < nkc:
                                emit_lg(j + LOOK)
                        drain(1)

                    # spill U' out of PSUM (frees pv banks), defer the rest
                    u_sbs = []
                    for h in range(2):
                        u_sb = epi.tile([DEPTH + 1, QT], F32, tag="u_sb",
                                        bufs=6)
                        nc.vector.tensor_copy(u_sb[:], pv_ps[h][:])
                        u_sbs.append(u_sb)
                    pending.extend(make_epi_chunks(pair, qt, u_sbs))
            drain(len(pending))
    nc.compile()
    return nc


_NC_CACHE = {}


def _get_program(key_args):
    if key_args not in _NC_CACHE:
        _NC_CACHE[key_args] = build_program(*key_args)
    return _NC_CACHE[key_args]


def make_in_maps(inputs, seq=S):
    """Host-side sharding: per-core input dicts."""
    try:
        import ml_dtypes
        bf16 = ml_dtypes.bfloat16
    except ImportError:
        bf16 = None

    def cast(x):
        return x.astype(bf16)

    q = np.asarray(inputs["q"], np.float32)
    k = np.asarray(inputs["k"], np.float32)
    v = np.asarray(inputs["v"], np.float32)
    Wq = np.asarray(inputs["Wq"], np.float32)
    Wk = np.asarray(inputs["Wk"], np.float32)
    Wv = np.asarray(inputs["Wv"], np.float32)
    Wo = np.asarray(inputs["Wo"], np.float32)
    bq = np.asarray(inputs["bq"], np.float32)
    bk = np.asarray(inputs["bk"], np.float32)
    bv = np.asarray(inputs["bv"], np.float32)

    qTb = [np.ascontiguousarray(q[b].T) for b in range(B)]
    kTb = [np.ascontiguousarray(k[b].T) for b in range(B)]
    vTb = [np.ascontiguousarray(v[b].T) for b in range(B)]

    in_maps = []
    for c in range(8):
        b, g = c // G, c % G
        cols = slice(g * DG, (g + 1) * DG)
        in_maps.append({
            "qT": cast(qTb[b]), "kT": cast(kTb[b]), "vT": cast(vTb[b]),
            "Wq": cast(np.ascontiguousarray(Wq[:, cols])),
            "Wk": cast(np.ascontiguousarray(Wk[:, cols])),
            "Wv": cast(np.ascontiguousarray(Wv[:, cols])),
            "Wo": cast(np.ascontiguousarray(Wo[cols, :])),
            "bq": np.ascontiguousarray(bq[cols].reshape(DG, 1)),
            "bk": np.ascontiguousarray(bk[cols].reshape(DG, 1)),
            "bv": cast(np.ascontiguousarray(bv[cols].reshape(1, DG))),
        })
    return in_maps


LAST_RESULT = None


def kernel(**inputs):
    global LAST_RESULT
    dve_num = int(os.environ.get("MHA_DVE_NUM", "2"))
    dve_den = int(os.environ.get("MHA_DVE_DEN", "5"))
    boff = float(os.environ.get("MHA_BOFF", "-7.4"))
    nc = _get_program((S, dve_num, dve_den, boff))
    in_maps = make_in_maps(inputs, S)
    res = run_bass_kernel_spmd(nc, in_maps, list(range(8)))
    LAST_RESULT = res
    bo = np.asarray(inputs["bo"], np.float32)
    out = np.zeros((B, S, D), np.float32)
    for c in range(8):
        b = c // G
        out[b] += res.results[c]["Y"]
    out += bo[None, None, :]
    return out


if __name__ == "__main__":
    # smoke build
    nc = build_program(1024)
    print("built ok")
